# revision 23
# baseline (speedup 1.0000x reference)
"""Trainium2 Bass kernel for nn_Discriminator_AddDim_ESSAAttn.

Network (per sample, C=128, 27x27 spatial, N=729 tokens):
  ESSA linear attention -> concat -> 1x1-conv FFN (+residual) ->
  3x3 conv/relu/pool x2 -> 3 FC layers -> [16] logits.
Batch 256 is sharded 32-per-core across 8 NeuronCores (pure data
parallel, weights replicated).

v2 layout strategy (vs v0 baseline at 587us):
  - w_ln folded into the ffn1 attn-weight host-side (attn = (v+t2)Wln
    only feeds ffn1, so W1a' = Wln @ W1a, b1' += W1a^T bln). Deletes the
    attn matmul + extract stage entirely.
  - q2-normalization algebra: q2/(sum+eps) then L2-normalize cancels the
    row scale, so q2n = q2 * rsqrt(sum(q2^2)). The whole s1q chain and
    the sq2 reduce are gone.
  - sq4 via a single bf16 TT square (gpsimd) + axis-reduce instead of 6
    STT + 6 accumulator reads.
  - per-token scales (cq, s1k) applied with ONE stride-0-broadcast
    tensor_tensor over all 6 token tiles instead of 6 tensor_scalars.
  - bf16 everywhere off the main residual path (q2/k2/kvsrc/q2n/
    transposes/kvsb/O2buf/fc weights): halves PE transpose cost, 2x-4x
    DVE modes, smaller DMA.
  - h extract fused: ACT Prelu(alpha=.01, bias=b1) in one instruction
    (parametric_relu lives in the same act table as sqrt/square).
  - conv extracts: raw copy psum->sbuf, maxpool on sbuf bf16, then ONE
    relu+bias on the pooled 144 cols (bias commutes with max).
  - single-instruction extracts for vt/q2nT/xen (cross-psum-bank APs).
  - kv psum: both pair samples in one bank [128,2,256]; diag of the k
    gram extracted with a fused tensor_tensor_reduce.
  - weight DMAs on the gpsimd queue (cheap dispatch), big fc weights
    emitted mid-loop so the first samples' x DMA isn't blocked.
  - 2-pair-deep software pipeline: ffn-chain stages of pair i interleave
    with conv stages of pair i-1 so each fills the other's psum-rotation
    stalls and the PE stays at the fast pstate.
"""
import sys

sys.path.insert(0, "/opt/trn_rl_repo")

import numpy as np

import concourse.bass as bass
import concourse.tile as tile
from concourse import mybir
from concourse.bass_utils import run_bass_kernel_spmd

F32 = mybir.dt.float32
F32R = mybir.dt.float32r
BF16 = mybir.dt.bfloat16
FP8 = mybir.dt.float8e4
AF = mybir.ActivationFunctionType
ALU = mybir.AluOpType
AX = mybir.AxisListType
DR = mybir.MatmulPerfMode.DoubleRow
W8SCALE = 64.0  # wqkv is ~N(0,.02^2): scale into fp8e4's normal range

N_CORES = 8
B, C, P = 256, 128, 27
NTOK = P * P          # 729
S = B // N_CORES      # 32 samples per core
NT = 6                # token tiles: 5*128 + 89
TOK_SIZES = [128, 128, 128, 128, 128, 89]
CGRP = 4              # conv2 sample-group size
CW = 26               # conv1 window width (25 valid + 1 pad; bf16 needs %2)


def _split_waits(nc, maxw=1):
    """walrus CoreV3 rejects instructions carrying >1 sem-wait; hoist
    extras onto preceding same-engine no-op carriers."""
    import bass_rust

    for bb in nc.m.functions[0].blocks:
        newlist = []
        for ins in bb.instructions:
            sw = ins.sync_info
            if sw and sw.on_wait and len(sw.on_wait) > maxw:
                waits = list(sw.on_wait)
                keep = waits[-maxw:]
                hoist = waits[:-maxw]
                for i in range(0, len(hoist), maxw):
                    chunk = hoist[i : i + maxw]
                    nop = bass_rust.InstNoOp(
                        name=f"{ins.name}_wsplit{i}", ins=[], outs=[]
                    )
                    nop.engine = ins.engine
                    nop.sync_info = mybir.SyncInfo(on_wait=list(chunk), on_update=[])
                    nc.register_instruction(nop, overwrite=True)
                    newlist.append(nop)
                ins.sync_info = mybir.SyncInfo(
                    on_wait=list(keep), on_update=list(sw.on_update)
                )
            newlist.append(ins)
        bb.instructions[:] = newlist


def _prep_weights(inputs):
    """Host-side weight massaging (all cheap numpy)."""
    f = lambda a: np.ascontiguousarray(np.asarray(a, np.float32))
    w_qkv = f(inputs["w_qkv"]).copy()          # [128, 384]
    b_qkv = f(inputs["b_qkv"]).copy()          # [384]
    # fold channel-mean subtraction of q and k into the weights/bias
    w_qkv[:, 0:128] -= w_qkv[:, 0:128].mean(axis=1, keepdims=True)
    w_qkv[:, 128:256] -= w_qkv[:, 128:256].mean(axis=1, keepdims=True)
    b_qkv[0:128] -= b_qkv[0:128].mean()
    b_qkv[128:256] -= b_qkv[128:256].mean()

    wln = f(inputs["w_ln"]).astype(np.float64)        # [128, 128]
    bln = f(inputs["b_ln"]).astype(np.float64)        # [128]
    w_ffn1 = f(inputs["w_ffn1"]).reshape(64, 256).astype(np.float64)  # [out, in]
    w1x = np.ascontiguousarray(w_ffn1[:, 0:128].T.astype(np.float32))   # [128, 64]
    w1a_raw = w_ffn1[:, 128:256].T                    # [128, 64] f64
    # attn = (v+t2) @ wln + bln only feeds ffn1: fold wln/bln in.
    w1a = np.ascontiguousarray((wln @ w1a_raw).astype(np.float32))      # [128, 64]
    b1 = f(inputs["b_ffn1"]).astype(np.float64) + w1a_raw.T @ bln       # [64]
    b1 = b1.astype(np.float32)
    w2t = np.ascontiguousarray(f(inputs["w_ffn2"]).reshape(128, 64).T)  # [64, 128]

    # conv taps -> [in_ch, 9, out_ch]
    wc1 = np.ascontiguousarray(
        f(inputs["w_c1"]).transpose(2, 3, 1, 0).reshape(9, 128, 64).transpose(1, 0, 2)
    )  # [128, 9, 64]
    wc2 = np.ascontiguousarray(
        f(inputs["w_c2"]).transpose(2, 3, 1, 0).reshape(9, 64, 128).transpose(1, 0, 2)
    )  # [64, 9, 128]

    w1r = np.ascontiguousarray(f(inputs["w_fc1"]).reshape(128, 25, 512))
    wf2 = np.ascontiguousarray(f(inputs["w_fc2"]).reshape(4, 128, 512).transpose(1, 0, 2))
    wcls = np.ascontiguousarray(f(inputs["w_cls"]).reshape(4, 128, 16).transpose(1, 0, 2))

    # fp8 DoubleRow layout: channel c -> (partition c%64, ktile c//64);
    # x64 scale keeps the ~N(0,.02^2) weights out of e4m3's subnormals.
    # The scale cancels in q2n (normalized) and k~ (eps *= 64^2), and the
    # 64x on v is what the wv8@x8 recompute produces anyway; one 1/64
    # activation-scale on the vt extract undoes it.
    wqkv8 = np.ascontiguousarray(
        (w_qkv * W8SCALE).reshape(2, 64, 384).transpose(1, 0, 2)
    )  # [64, 2, 384]

    col = lambda a: np.ascontiguousarray(f(a).reshape(-1, 1))
    row = lambda a: np.ascontiguousarray(f(a).reshape(1, -1))
    w = {
        "wqkv8": wqkv8,
        "bqkv_row": row(b_qkv * W8SCALE),
        "w1x": w1x,
        "w1a": w1a,
        "b1": col(b1),
        "w2t": w2t,
        "b2": col(inputs["b_ffn2"]),
        "wc1": wc1,
        "bc1": col(inputs["b_c1"]),
        "wc2": wc2,
        "bc2": col(inputs["b_c2"]),
        "w1r": w1r,
        "b1row": row(inputs["b_fc1"]),
        "wf2": wf2,
        "b2row": row(inputs["b_fc2"]),
        "wcls": wcls,
        "bcrow": row(inputs["b_cls"]),
        "eye": np.eye(128, dtype=np.float32),
        "eyebf": np.eye(128, dtype=np.float32),
        "ones1": np.ones((1, S), dtype=np.float32),
    }
    flags = {
        "qkv_bias": bool(np.any(b_qkv)),
        "fc1_bias": bool(np.any(w["b1row"])),
        "fc2_bias": bool(np.any(w["b2row"])),
        "cls_bias": bool(np.any(w["bcrow"])),
    }
    return w, flags


class _W:
    pass


_F32_WEIGHTS = {"b1", "b2", "bc1", "bc2", "eye"}  # act-bias operands + f32 eye
_BF16_WEIGHTS = {"w1a", "w2t", "wc1", "wc2", "w1r", "wf2", "wcls", "eyebf"}
_FP8_WEIGHTS = {"wqkv8"}
_LATE_WEIGHTS = {"w1r", "wf2", "wcls", "b1row", "b2row", "bcrow"}  # fc-only


def _load_weights(nc, pool, wvals, names):
    """Declare dram params + DMA weights into resident SBUF tiles on the
    gpsimd DGE queue (cheap dispatch; casts bf16 in flight)."""
    W = _W()
    for name in names:
        arr = wvals[name]
        if name in _F32_WEIGHTS:
            dt = F32
        elif name in _BF16_WEIGHTS:
            dt = BF16
        elif name in _FP8_WEIGHTS:
            dt = FP8
        else:
            dt = F32R
        dram = nc.declare_dram_parameter(
            name, list(arr.shape), F32 if dt in (BF16, FP8) else dt, isOutput=False
        )
        t = pool.tile(list(arr.shape), dt, name=f"sb_{name}")
        nc.gpsimd.dma_start(out=t, in_=dram[:])
        setattr(W, name, t)
    return W


def _win(ap, offset, dims):
    """Manual sub-AP of a tile: dims = [[stride, count], ...] free dims."""
    return bass.AP(
        tensor=ap.tensor, offset=ap.offset + offset,
        ap=[list(ap.ap[0])] + [list(d) for d in dims],
    )


def _bcast(ap, n):
    """Append a stride-0 innermost dim of size n (broadcast read)."""
    return bass.AP(
        tensor=ap.tensor, offset=ap.offset,
        ap=[list(d) for d in ap.ap] + [[0, n]],
    )


def _mm(nc, out, lhsT, rhs, start=True, stop=True):
    nc.tensor.matmul(out, lhsT, rhs, start=start, stop=stop)


def _tp(nc, out, in_, eye):
    nc.tensor.matmul(
        out.bitcast(in_.dtype), in_, eye.bitcast(in_.dtype), is_transpose=True
    )


def _s0(nc, pools, W, flags, x_dram, x8_dram, s, taps):
    """x DMA + qkv matmuls (fp8 DoubleRow) + extracts + token stats.
    Leaves q2n (bf16, normalized) and kvsrc (64v | k~) ready."""
    acts, stats, psum = pools["acts"], pools["stats"], pools["psum"]
    st = {"s": s}

    x_s = acts.tile([C, 768], F32R, name="x_s", bufs=6)
    nc.sync.dma_start(out=x_s[:, 0:NTOK], in_=x_dram[s])
    st["x_s"] = x_s
    x8 = acts.tile([64, 2, 768], FP8, name="x8", bufs=6)
    nc.sync.dma_start(out=x8[:, :, 0:NTOK], in_=x8_dram[s])
    st["x8"] = x8

    q2 = acts.tile([128, NT, 128], BF16, name="q2", bufs=2)
    k2 = acts.tile([128, NT, 128], BF16, name="k2", bufs=2)
    kvsrc = acts.tile([128, NT, 256], BF16, name="kvsrc", bufs=4)
    for half in range(2):
        pq = psum.tile([128, 3, 512], F32, name="pq", tag="pq", bufs=1)
        for i in range(3):
            t = half * 3 + i
            nt = TOK_SIZES[t]
            nc.tensor.matmul(
                pq[0:nt, i, 0:384], x8[:, :, 128 * t : 128 * t + nt], W.wqkv8,
                start=True, stop=not flags["qkv_bias"], perf_mode=DR,
            )
            if flags["qkv_bias"]:
                _mm(nc, pq[0:nt, i, 0:384], W.ones1[0:1, 0:nt], W.bqkv_row,
                    start=False, stop=True)
        h3 = slice(3 * half, 3 * half + 3)
        nc.scalar.activation(q2[:, h3, :], pq[:, :, 0:128], AF.Square)
        nc.scalar.activation(k2[:, h3, :], pq[:, :, 128:256], AF.Square)
        nc.vector.tensor_copy(kvsrc[:, h3, 0:128], pq[:, :, 256:384])

    # q-side: q2n = q2 * rsqrt(sum(q2^2)); the row-sum scale AND the 64^2
    # fp8 weight scale both cancel in the L2 normalization.
    q4 = acts.tile([128, NT, 128], BF16, name="q4")
    nc.gpsimd.tensor_mul(q4, q2, q2)
    sq4 = stats.tile([128, NT], F32, name="sq4")
    nc.vector.reduce_sum(sq4, q4, axis=AX.X)
    nq = stats.tile([128, NT], F32, name="nq")
    nc.scalar.activation(nq, sq4, AF.Sqrt)
    cq = stats.tile([128, NT], F32, name="cq")
    nc.vector.tensor_scalar_max(cq, nq, 1e-20)
    nc.vector.reciprocal(cq, cq)
    cqb = stats.tile([128, NT], BF16, name="cqb")
    nc.vector.tensor_scalar_mul(cqb, cq, 1.0)
    q2n = acts.tile([128, NT, 128], BF16, name="q2n", bufs=4)
    nc.gpsimd.tensor_tensor(out=q2n, in0=q2, in1=_bcast(cqb, 128), op=ALU.mult)

    # k-side: k~ = k2 / (sum(k2) + eps); k2 carries the 64^2 weight scale
    # so eps scales along with it and k~ comes out exactly right.
    sk2 = stats.tile([128, NT], F32, name="sk2")
    nc.vector.reduce_sum(sk2, k2, axis=AX.X)
    s1k = stats.tile([128, NT], F32, name="s1k")
    nc.vector.tensor_scalar_add(s1k, sk2, 1e-7 * W8SCALE * W8SCALE)
    nc.vector.reciprocal(s1k, s1k)
    s1kb = stats.tile([128, NT], BF16, name="s1kb")
    nc.vector.tensor_scalar_mul(s1kb, s1k, 1.0)
    nc.gpsimd.tensor_tensor(
        out=kvsrc[:, :, 128:256], in0=k2, in1=_bcast(s1kb, 128), op=ALU.mult
    )
    st.update(q2n=q2n, kvsrc=kvsrc, q2=q2)
    return st


def _s1_kv_mm(nc, pools, W, sts):
    """kv gram matmuls for the pair: both samples in one psum bank."""
    psum = pools["psum"]
    pkv = psum.tile([128, 2, 256], F32, name="pkv", tag="ps1", bufs=1)
    for j, st in enumerate(sts):
        kvsrc = st["kvsrc"]
        for t in range(NT):
            nt = TOK_SIZES[t]
            _mm(nc, pkv[:, j, 0:256], kvsrc[0:nt, t, 128:256], kvsrc[0:nt, t, :],
                start=(t == 0), stop=(t == NT - 1))
        st["pkv"] = pkv[:, j, :]


def _s1_kv_post(nc, pools, W, st):
    """Extract kv, normalized by the k-gram diag (col norms of k~)."""
    acts, stats = pools["acts"], pools["stats"]
    pkv = st["pkv"]
    junk = acts.tile([128, 128], F32, name="junk")
    s2 = stats.tile([128, 1], F32, name="s2")
    nc.vector.tensor_mul(junk, pkv[:, 128:256], W.eye)
    nc.vector.reduce_sum(s2, junk, axis=AX.X)
    invs = stats.tile([128, 1], F32, name="invs")
    nc.scalar.activation(invs, s2, AF.Sqrt, scale=float(NTOK))  # 27*sqrt(s2)
    nc.vector.tensor_scalar_max(invs, invs, 27e-12)
    nc.vector.reciprocal(invs, invs)
    kvsb = acts.tile([128, 128], BF16, name="kvsb")
    nc.scalar.mul(kvsb, pkv[:, 0:128], invs)
    st["kvsb"] = kvsb


def _s1_tp(nc, pools, W, st):
    """PE-transpose q2n (bf16, bitcast-packed psum) + extract."""
    acts, psum = pools["acts"], pools["psum"]
    q2n = st["q2n"]
    pqt = psum.tile([128, NT, 64], F32, name="pqt", tag="pq", bufs=1)
    pqtv = pqt.bitcast(BF16)  # [128, NT, 128]
    for t in range(NT):
        _tp(nc, pqtv[:, t, :], q2n[:, t, :], W.eyebf)
    q2nT = acts.tile([128, 768], BF16, name="q2nT")
    nc.scalar.copy(q2nT, pqtv.rearrange("p a b -> p (a b)"))
    st["q2nT"] = q2nT


def _s1_t2(nc, pools, W, st):
    """vt = v + t2 (64v recomputed wv8@x8; kvsb also carries the 64x, so
    one 1/64 activation-scale on the extract restores true scale)."""
    acts, psum = pools["acts"], pools["psum"]
    x8, q2nT, kvsb = st["x8"], st["q2nT"], st["kvsb"]
    wv8 = W.wqkv8[:, :, 256:384]
    pt2 = psum.tile([128, 768], F32, name="pt2", tag="ps2", bufs=1)
    nc.tensor.matmul(pt2[:, 0:512], wv8, x8[:, :, 0:512],
                     start=True, stop=False, perf_mode=DR)
    nc.tensor.matmul(pt2[:, 512:768], wv8, x8[:, :, 512:768],
                     start=True, stop=False, perf_mode=DR)
    _mm(nc, pt2[:, 0:512], kvsb, q2nT[:, 0:512], start=False, stop=True)
    _mm(nc, pt2[:, 512:768], kvsb, q2nT[:, 512:768], start=False, stop=True)
    vt = acts.tile([C, 732], BF16, name="vt", bufs=2)
    nc.scalar.mul(vt[:, 0:NTOK], pt2[:, 0:NTOK], 1.0 / W8SCALE)
    st["vt"] = vt


def _s2_ffn1(nc, pools, W, st):
    """h = prelu(w1x@x + w1a'@vt + b1', slope .01) -- wln pre-folded."""
    acts, psum = pools["acts"], pools["psum"]
    x_s, vt = st["x_s"], st["vt"]
    ph = psum.tile([64, 768], F32, name="ph", tag="ps2", bufs=1)
    _mm(nc, ph[:, 0:512], W.w1x, x_s[:, 0:512], start=True, stop=False)
    _mm(nc, ph[:, 512:768], W.w1x, x_s[:, 512:768], start=True, stop=False)
    _mm(nc, ph[:, 0:512], W.w1a, vt[:, 0:512], start=False, stop=True)
    _mm(nc, ph[:, 512:732], W.w1a, vt[:, 512:732], start=False, stop=True)
    h = acts.tile([64, 732], BF16, name="h", bufs=2)
    nc.scalar.activation(h[:, 0:NTOK], ph[:, 0:NTOK], AF.Prelu,
                         bias=W.b1, alpha=0.01)
    st["h"] = h


def _s2_ffn2(nc, pools, W, st):
    acts, psum = pools["acts"], pools["psum"]
    x_s, h = st["x_s"], st["h"]
    pxen = psum.tile([128, 768], F32, name="pxen", tag="ps2", bufs=1)
    _mm(nc, pxen[:, 0:512], W.w2t, h[:, 0:512], start=True, stop=True)
    _mm(nc, pxen[:, 512:732], W.w2t, h[:, 512:732], start=True, stop=True)
    xen = acts.tile([C, 768], BF16, name="xen", bufs=6)
    # residual + bias folded into the extract
    nc.vector.scalar_tensor_tensor(
        out=xen[:, 0:NTOK], in0=pxen[:, 0:NTOK], scalar=W.b2, in1=x_s[:, 0:NTOK],
        op0=ALU.add, op1=ALU.add,
    )
    st["xen"] = xen


def _s2_c1(nc, pools, W, st, grp, taps):
    """conv1 (9 accumulating taps, 26-wide windows) -> raw extract ->
    2x2 maxpool on sbuf -> one relu+bias into the o1p group tile."""
    acts, psum = pools["acts"], pools["psum"]
    xen, s = st["xen"], st["s"]
    pc1 = psum.tile([64, 2, 512], F32, name="pc1", tag="psc1", bufs=1)
    for ky in range(3):
        for kx in range(3):
            tap = ky * 3 + kx
            # rows 0:12 -> half 0, rows 12:25 -> half 1 (pool pairs never
            # straddle the split; conv row 24 is dropped by the pool)
            _mm(nc, pc1[:, 0, 0 : 12 * CW].rearrange("p (a b) -> p a b", a=12),
                W.wc1[:, tap, :],
                _win(xen, ky * 27 + kx, [[27, 12], [1, CW]]),
                start=(tap == 0), stop=(tap == 8))
            _mm(nc, pc1[:, 1, 0 : 13 * CW].rearrange("p (a b) -> p a b", a=13),
                W.wc1[:, tap, :],
                _win(xen, (ky + 12) * 27 + kx, [[27, 13], [1, CW]]),
                start=(tap == 0), stop=(tap == 8))
    o1r = acts.tile([64, 2, 13, CW], BF16, name="o1r")
    nc.scalar.copy(
        o1r,
        _win(pc1, 0, [[512, 2], [CW, 13], [1, CW]]),
    )
    # maxpool 25x25 -> 12x12 (row 24 / col 24 dropped), bias+relu after
    m1 = acts.tile([64, 2, 6, CW], BF16, name="m1")
    nc.vector.tensor_max(m1[:, 0], o1r[:, 0, 0:12:2, :], o1r[:, 0, 1:12:2, :])
    nc.vector.tensor_max(m1[:, 1], o1r[:, 1, 0:12:2, :], o1r[:, 1, 1:12:2, :])
    o1pre = acts.tile([64, 12, 12], BF16, name="o1pre")
    nc.vector.tensor_max(
        o1pre.rearrange("p (ha a) b -> p ha a b", ha=2),
        m1[:, :, :, 0:24:2], m1[:, :, :, 1:25:2],
    )
    g = s % CGRP
    o1pv = grp["o1p"][:, g, 0:144].rearrange("p (a b) -> p a b", a=12)
    nc.scalar.activation(o1pv, o1pre, AF.Relu, bias=W.bc1)

    if taps is not None and s == 0:
        for nm, t in (
            ("q2n", st["q2n"]), ("kvsb", st["kvsb"]), ("xen", xen),
            ("o1p", grp["o1p"][:, 0, :]), ("vt", st["vt"]), ("q2", st["q2"]),
            ("h", st["h"]), ("q2nT", st["q2nT"]),
        ):
            d = nc.declare_dram_parameter(f"tap_{nm}", list(t.shape), t.dtype, isOutput=True)
            nc.sync.dma_start(out=d[:], in_=t)
            taps.append(f"tap_{nm}")


def _emit_conv2_group(nc, pools, W, O2buf, grp, g0, gn):
    """conv2+pool for a group of gn samples (10-wide windows, no pad)."""
    acts, psum = pools["acts"], pools["psum"]
    pc2 = psum.tile([128, CGRP, 10, 10], F32, name="pc2", tag="ps1", bufs=1)
    for ky in range(3):
        for kx in range(3):
            tap = ky * 3 + kx
            _mm(nc, pc2[:, 0:gn], W.wc2[:, tap, :],
                _win(grp["o1p"], ky * 12 + kx, [[148, gn], [12, 10], [1, 10]]),
                start=(tap == 0), stop=(tap == 8))
    o2r = acts.tile([128, CGRP, 100], BF16, name="o2r")
    o2rv = o2r.rearrange("p g (h w) -> p g h w", h=10)
    nc.scalar.copy(o2rv[:, 0:gn], pc2[:, 0:gn])
    n1 = acts.tile([128, CGRP, 5, 10], BF16, name="n1")
    nc.vector.tensor_max(
        n1[:, 0:gn], o2rv[:, 0:gn, 0:10:2, :], o2rv[:, 0:gn, 1:10:2, :]
    )
    n2 = acts.tile([128, CGRP, 5, 5], BF16, name="n2")
    nc.vector.tensor_max(
        n2[:, 0:gn], n1[:, 0:gn, :, 0:10:2], n1[:, 0:gn, :, 1:10:2]
    )
    outv = (
        O2buf[:, :, g0 : g0 + gn]
        .rearrange("p a g -> p g a")
        .rearrange("p g (a b) -> p g a b", a=5)
    )
    nc.scalar.activation(outv, n2[:, 0:gn], AF.Relu, bias=W.bc2)


def _emit_fc(nc, pools, W, flags, out_dram, O2buf, ns):
    psum, fc = pools["psum"], pools["fc"]
    ones = W.ones1[0:1, 0:ns]

    po3 = psum.tile([ns, 512], F32, name="po3", tag="ps1", bufs=1)
    for p in range(25):
        _mm(nc, po3, O2buf[:, p, :], W.w1r[:, p, :],
            start=(p == 0), stop=(p == 24 and not flags["fc1_bias"]))
    if flags["fc1_bias"]:
        _mm(nc, po3, ones, W.b1row, start=False, stop=True)
    o3r = fc.tile([ns, 512], BF16, name="o3r")
    nc.scalar.activation(o3r, po3, AF.Relu)

    po3t = psum.tile([128, 4, ns // 2], F32, name="po3t", tag="ps1", bufs=1)
    po3tv = po3t.bitcast(BF16)
    for j in range(4):
        _tp(nc, po3tv[:, j, :], o3r[:, 128 * j : 128 * (j + 1)], W.eyebf[0:ns, 0:ns])
    o3T = fc.tile([128, 4, ns], BF16, name="o3T")
    nc.vector.tensor_copy(o3T, po3tv)

    po4 = psum.tile([ns, 512], F32, name="po4", tag="ps1", bufs=1)
    for j in range(4):
        _mm(nc, po4, o3T[:, j, :], W.wf2[:, j, :],
            start=(j == 0), stop=(j == 3 and not flags["fc2_bias"]))
    if flags["fc2_bias"]:
        _mm(nc, po4, ones, W.b2row, start=False, stop=True)
    o4r = fc.tile([ns, 512], BF16, name="o4r")
    nc.scalar.activation(o4r, po4, AF.Relu)

    po4t = psum.tile([128, 4, ns // 2], F32, name="po4t", tag="ps1", bufs=1)
    po4tv = po4t.bitcast(BF16)
    for j in range(4):
        _tp(nc, po4tv[:, j, :], o4r[:, 128 * j : 128 * (j + 1)], W.eyebf[0:ns, 0:ns])
    o4T = fc.tile([128, 4, ns], BF16, name="o4T")
    nc.vector.tensor_copy(o4T, po4tv)

    pcls = psum.tile([ns, 512], F32, name="pcls", tag="ps1", bufs=1)
    for j in range(4):
        _mm(nc, pcls[:, 0:16], o4T[:, j, :], W.wcls[:, j, :],
            start=(j == 0), stop=(j == 3 and not flags["cls_bias"]))
    if flags["cls_bias"]:
        _mm(nc, pcls[:, 0:16], ones, W.bcrow, start=False, stop=True)
    outsb = fc.tile([ns, 16], F32, name="outsb")
    nc.vector.tensor_copy(outsb, pcls[:, 0:16])
    nc.sync.dma_start(out=out_dram[:], in_=outsb)


_EARLY_WEIGHTS = [
    "wqkv8", "bqkv_row", "eye", "eyebf", "ones1", "w1x", "w1a", "b1",
    "w2t", "b2", "wc1", "bc1", "wc2", "bc2",
]


def build_nc(wvals, flags, n_samples=S, debug=False):
    nc = bass.Bass()
    x_dram = nc.declare_dram_parameter("x", [n_samples, C, NTOK], F32R, isOutput=False)
    x8_dram = nc.declare_dram_parameter(
        "x8", [n_samples, 64, 2, NTOK], FP8, isOutput=False
    )
    out_dram = nc.declare_dram_parameter("out", [n_samples, 16], F32, isOutput=True)
    taps = [] if debug else None

    with tile.TileContext(nc) as tc:
        with (
            tc.tile_pool(name="wts", bufs=1) as wts,
            tc.tile_pool(name="acts", bufs=2) as acts,
            tc.tile_pool(name="stats", bufs=3) as stats,
            tc.tile_pool(name="fc", bufs=1) as fc,
            tc.tile_pool(name="psum", bufs=1, space="PSUM") as psum,
        ):
            pools = {"acts": acts, "stats": stats, "psum": psum, "fc": fc}
            W = _load_weights(nc, wts, wvals, _EARLY_WEIGHTS)
            O2buf = fc.tile([128, 25, n_samples], BF16, name="O2buf")
            grp = {}  # group-index -> {"o1p": tile}

            def ffn_stages(sts):
                # kv gram / transpose / t2 / ffn for one pair
                _s1_kv_mm(nc, pools, W, sts)
                for st in sts:
                    _s1_kv_post(nc, pools, W, st)
                for st in sts:
                    _s1_tp(nc, pools, W, st)
                for st in sts:
                    _s1_t2(nc, pools, W, st)
                for st in sts:
                    _s2_ffn1(nc, pools, W, st)
                for st in sts:
                    _s2_ffn2(nc, pools, W, st)

            def conv_stages(sts):
                for st in sts:
                    _s2_c1(nc, pools, W, st, grp[st["s"] // CGRP], taps)
                s_last = sts[-1]["s"]
                if s_last % CGRP == CGRP - 1 or s_last == n_samples - 1:
                    g0 = (s_last // CGRP) * CGRP
                    _emit_conv2_group(
                        nc, pools, W, O2buf, grp[g0 // CGRP], g0, s_last - g0 + 1
                    )

            # 2-pair-deep pipeline: pair i's qkv is emitted first, then
            # pair i-1's ffn chain, then pair i-2's convs -- the conv
            # matmuls fill the ffn chain's psum-rotation stalls.
            pend_ffn = None   # pair awaiting ffn stages
            pend_conv = None  # pair awaiting conv stages
            pairs = [
                list(range(p0, min(p0 + 2, n_samples)))
                for p0 in range(0, n_samples, 2)
            ]
            for idx, pair in enumerate(pairs):
                sts = []
                for s in pair:
                    if s % CGRP == 0:
                        grp[s // CGRP] = {
                            "o1p": acts.tile([64, CGRP, 148], BF16, name="o1p_grp", bufs=3)
                        }
                    sts.append(_s0(nc, pools, W, flags, x_dram, x8_dram, s, taps))
                if idx == 1:
                    # big fc weights: queue behind the first x loads
                    Wl = _load_weights(nc, wts, wvals, sorted(_LATE_WEIGHTS))
                    for nm in _LATE_WEIGHTS:
                        setattr(W, nm, getattr(Wl, nm))
                if pend_ffn is not None:
                    ffn_stages(pend_ffn)
                if pend_conv is not None:
                    conv_stages(pend_conv)
                pend_conv = pend_ffn
                pend_ffn = sts
            ffn_stages(pend_ffn)
            conv_stages(pend_conv)
            conv_stages(pend_ffn)
            _emit_fc(nc, pools, W, flags, out_dram, O2buf, n_samples)

    _split_waits(nc)
    return nc, taps


_BUILD_CACHE = {}


def make_in_maps(inputs, wvals):
    import ml_dtypes

    x = np.ascontiguousarray(np.asarray(inputs["x"], np.float32)).reshape(
        N_CORES, S, C, NTOK
    )
    # fp8 copy in DoubleRow layout: channel c -> (partition c%64, ktile c//64)
    x8 = np.ascontiguousarray(
        x.reshape(N_CORES, S, 2, 64, NTOK).transpose(0, 1, 3, 2, 4)
    ).astype(ml_dtypes.float8_e4m3)
    in_maps = []
    for c in range(N_CORES):
        m = {"x": np.ascontiguousarray(x[c]), "x8": np.ascontiguousarray(x8[c])}
        m.update(wvals)
        in_maps.append(m)
    return in_maps


def kernel(**inputs):
    wvals, flags = _prep_weights(inputs)
    key = tuple(sorted(flags.items()))
    if key not in _BUILD_CACHE:
        _BUILD_CACHE[key] = build_nc(wvals, flags)
    nc, _ = _BUILD_CACHE[key]

    in_maps = make_in_maps(inputs, wvals)
    last_err = None
    for _attempt in range(3):
        try:
            res = run_bass_kernel_spmd(nc, in_maps, core_ids=list(range(N_CORES)))
            break
        except Exception as e:  # transient device faults: retry
            last_err = e
    else:
        raise last_err
    out = np.concatenate([res.results[c]["out"] for c in range(N_CORES)], axis=0)
    return out.astype(np.float32)
